# revision 38
# baseline (speedup 1.0000x reference)
"""Trainium2 Bass kernel for nn_Attn_6545530159401.

Computation (reference):
    enc  = encoder_outputs.transpose(1,0,2)            # (B,T,H)
    cat  = concat([hidden broadcast, enc], -1)         # (B,T,2H)
    en   = tanh(cat @ W_attn.T + b_attn)               # (B,T,H)
    sc   = en @ v                                      # (B,T)
    out  = softmax(sc, axis=1)[:, None, :]             # (B,1,T)

Split W_attn = [W_h | W_e] (each (H,H)):
    q[b]     = hidden[b] @ W_h.T + b_attn              # (B,H) tiny
    E[b,t]   = enc[b,t] @ W_e.T                        # the big matmul
    sc[b,t]  = sum_o v[o] * tanh(q[b,o] + E[b,t,o])

Sharding: data-parallel over B across 8 NeuronCores (4 batches/core),
no collectives. Per-core pipeline (o-chunks on PSUM partitions so q can
ride the ACT bias port):
    mains   PE: E-psum (128 o, 1024 rows) in fp8e4 DoubleRow mode
            (2 MACs/cell/cycle): weights host-scaled by 32 into e4m3,
            enc in e4m3, k-chunk pairs contracted 256-at-a-time; the
            1/32 descale rides the tanh activation's scale port
    tanh    ACT: tanh(E/32 + q[b, o-chunk]) via per-partition bias
    z       DVE: z(s) = sum_o v_o * tanh_o via fused scalar_tensor_tensor
            (DVE is ~25% busy — ACT is the steady-state limiter, so the
            fewer-instruction chain wins over 2x-mode alternatives)
    scores  PE: sliding-window ones lhsT contracts z's 128 partitions
            into row 2s+h2 of one persistent (16, 512) psum tile
            (last superblock v-dots straight from tanh so the tail never
            waits on the z chain); emissions pipelined behind so the
            in-order PE never waits on ACT/DVE/Pool
    softmax ACT exp with fused accum sum on (16, 512), per-batch
            sum/broadcast via tiny PE matmuls, no max-subtraction
            (scores are bounded by ||v||_1)

DMA: enct is packed so each (partition, superblock) slab is one
contiguous 4KB run; small constants ride one combined `head` param
(single semaphore — separate small transfers straggled behind the enc
flood at packet round-robin). Rings: ACT HWDGE carries head+wetp only
(weights needed first, and ACT must not spend cycles on enc descriptor
emission), SP HWDGE carries e0..e3 (e0 split per DoubleRow k-pair so
the first matmul starts after 256KB), Pool SWDGE carries whtp8+e4..e7.
The q path is fp8 (whtp8 scaled by 32, hidden as e4m3 in head; the
descale rides a DVE scalar_tensor_tensor against the bias add).
8 warmup matmuls on a zeroed tile cover a full 4096-cycle HAM window
so the PE clock gate is released (2.4 GHz) when real matmuls start.
"""

import numpy as np
from contextlib import ExitStack

import ml_dtypes

import concourse.bass as bass
import concourse.tile as tile
from concourse import bacc, mybir
import concourse.bass_utils as bass_utils

T, B, H = 2048, 32, 512
NCORES = 8
NB = B // NCORES        # 4 local batches per core
ROWS = NB * T           # 8192 rows per core
P = 128
KC = H // P             # 4 contraction chunks
KP = KC // 2            # 2 DoubleRow k-chunk pairs
OC = H // P             # 4 output chunks
SUP = 1024              # columns per E-psum tile (2 PSUM banks)
NSUP = ROWS // SUP      # 8
BLK = 512               # matmul moving-dim limit (one PSUM bank)
WS = 32.0               # host-side fp8 weight scale (descale in tanh)
F32 = mybir.dt.float32
F16 = mybir.dt.float16
F8 = mybir.dt.float8e4
NP_F8 = ml_dtypes.float8_e4m3
AF = mybir.ActivationFunctionType
DR = mybir.MatmulPerfMode.DoubleRow


def _build():
    nc = bacc.Bacc(
        "TRN2", target_bir_lowering=False, debug=False, num_devices=NCORES
    )
    # enct[p, s, k*SUP+j] = enc^T[k*128+p, s*SUP+j] (e4m3): one
    # contiguous 4KB run per (partition, superblock) — declared FLAT in
    # the free dim so the DGE emits one 4KB descriptor per partition
    # (a 3D [P,KC,SUP] access pattern emits 4x 1KB descriptors and the
    # transfer runs descriptor-bound)
    enct = nc.declare_dram_parameter(
        "enct", [P, NSUP, KC * SUP], F8, isOutput=False
    )
    # wetp[p, k*H+o] = 32 * W_e^T[k*128+p, o] (e4m3), flat for the DGE
    wetp = nc.declare_dram_parameter("wetp", [P, KC * H], F8, isOutput=False)
    # whtp8[p, k*H+o] = 32 * W_h^T[k*128+p, o] (e4m3)
    whtp8 = nc.declare_dram_parameter("whtp8", [P, KC * H], F8, isOutput=False)
    # head: all small constants in one param (f32 view (P, 132)):
    #   f32 cols  0:16  brep[p, o*NB+b] = b_attn[o*128+p]
    #   f32 cols 16:20  vp[p, o] = v[o*128+p]
    #   f32 cols 20:24  selb2 (rows 0:16)
    #   f32 cols 24:40  selb  (rows 0:4)
    #   f32 cols 40:56  onesw as f16[32]: ones at f16-col 15 (sliding lhsT)
    #   f32 cols 64:128 vwin: 4 chunks of 32 f16 cols, chunk o has
    #                   v[o*128+p] at f16-col 128+32*o+15 (sliding lhsT);
    #                   f16 col 159 (unused by the windows) holds 1/32
    #   f32 cols 128:132  hid8[p, k*NB+b] = e4m3(hidden[b, k*128+p])
    head = nc.declare_dram_parameter("head", [P, 132], F32, isOutput=False)
    out = nc.declare_dram_parameter("out", [NB, T], F32, isOutput=True)

    with tile.TileContext(nc) as tc, ExitStack() as ctx:
        const_pool = ctx.enter_context(tc.tile_pool(name="const", bufs=1))
        enc_pool = ctx.enter_context(tc.tile_pool(name="enc", bufs=1))
        tanh_pool = ctx.enter_context(tc.tile_pool(name="tanh", bufs=6))
        sm_pool = ctx.enter_context(tc.tile_pool(name="sm", bufs=1))
        psE_pool = ctx.enter_context(tc.tile_pool(name="psE", bufs=3, space="PSUM"))
        psS_pool = ctx.enter_context(tc.tile_pool(name="psS", bufs=1, space="PSUM"))

        # ACT HWDGE ring: the head constants (q path inputs — needed so q
        # can be computed during the warmup window), then the E weights;
        # keeps ACT free of enc descriptors
        head_sb = const_pool.tile([P, 132], F32, tag="head")
        nc.scalar.dma_start(head_sb[:], head[:, :])
        wetp_sb = const_pool.tile([P, KC, H], F8, tag="wetp")
        nc.scalar.dma_start(
            wetp_sb[:].rearrange("p k o -> p (k o)"), wetp[:, :]
        )

        # SP HWDGE ring: first enc superblock split per DoubleRow k-pair
        # so the first matmul starts after 256KB, then e1..e3
        enc_sb = [None] * NSUP
        e0 = enc_pool.tile([P, KC, SUP], F8, tag="e0", name="e0")
        for kp in range(KP):
            nc.sync.dma_start(
                e0[:, 2 * kp : 2 * kp + 2, :].rearrange("p k t -> p (k t)"),
                enct[:, 0, kp * 2 * SUP : (kp + 1) * 2 * SUP],
            )
        warm = const_pool.tile([P, BLK], F16, tag="warm")
        nc.gpsimd.memset(warm[:], 0.0)
        enc_sb[0] = e0

        def load_sup(s, engine):
            e = enc_pool.tile([P, KC, SUP], F8, tag=f"e{s}", name=f"e{s}")
            engine.dma_start(
                e[:].rearrange("p k t -> p (k t)"), enct[:, s, :]
            )
            return e

        for s in range(1, NSUP // 2):
            enc_sb[s] = load_sup(s, nc.sync)

        # Pool SWDGE queue: q weights, then the back half of enc behind
        # an artificial dependency on e0 so the 2MB tail doesn't steal
        # HBM/descriptor bandwidth from the critical-path head transfers
        whtp8_sb = const_pool.tile([P, KC, H], F8, tag="whtp8")
        nc.gpsimd.dma_start(
            whtp8_sb[:].rearrange("p k o -> p (k o)"), whtp8[:, :]
        )
        gate = const_pool.tile([1, 1], F32, tag="gate")
        nc.gpsimd.tensor_copy(gate[:], e0[0:1, 0, 0:4].bitcast(F32))
        for s in range(NSUP // 2, NSUP):
            enc_sb[s] = load_sup(s, nc.gpsimd)

        c16 = head_sb[:, 0:128].bitcast(F16)  # (P, 256) f16 view
        brep_sb = head_sb[:, 0:16]
        vp_sb = head_sb[:, 16:20]
        selb2_sb = head_sb[0:16, 20:24]
        selb_sb = head_sb[0:4, 24:40]
        onesw_sb = c16[:, 80:111]
        vwin_sb = [c16[:, 128 + 32 * o : 159 + 32 * o] for o in range(OC)]
        inv32_sb = c16[:, 159:160]
        hid8 = head_sb[:, 128:132].bitcast(F8)  # (P, 16) f8 view
        hid_sb = [hid8[:, NB * k : NB * (k + 1)] for k in range(KC)]

        # PE warmup: 11 matmuls on a zeroed scratch tile run while the
        # first DMAs are still in flight, bridging to the first real
        # matmul so the PE is continuously busy for a full (free-running)
        # 4096-cycle HAM window and the clock gate releases to 2.4 GHz.
        # (8 warmups = 3.41us could straddle two windows and leave
        # neither fully busy — observed unthrottle as late as 20us.)
        psW = psS_pool.tile([P, BLK], F32, tag="t", name="psW")
        for _ in range(14):
            nc.tensor.matmul(
                psW[:], lhsT=warm[:, 0:P], rhs=warm[:], start=True, stop=True
            )

        # q[o, oc*4+b] = sum_h hid[h,b] * wht[h,o] + b_attn[o], fp8 with
        # the /32 descale fused into the bias add. Runs right after the
        # warmups — still inside the e0 DMA window — so the first tanh
        # never waits on the q chain.
        q_sb = const_pool.tile([P, OC * NB], F32, tag="q")
        psq = psS_pool.tile([P, OC * NB], F32, tag="t", name="psq")
        for o in range(OC):
            for k in range(KC):
                nc.tensor.matmul(
                    psq[:, o * NB : (o + 1) * NB],
                    lhsT=whtp8_sb[:, k, o * P : (o + 1) * P],
                    rhs=hid_sb[k][:],
                    start=(k == 0),
                    stop=(k == KC - 1),
                )
        nc.vector.scalar_tensor_tensor(
            q_sb[:],
            psq[:],
            inv32_sb[:],
            brep_sb[:],
            op0=mybir.AluOpType.mult,
            op1=mybir.AluOpType.add,
        )

        # single persistent score accumulator: row 2s+h2 = scores of
        # (batch s//2, t-slice (s%2)*1024 + h2*512)
        psS16 = psS_pool.tile([16, BLK], F32, tag="s16", name="psS16")

        # z(s) = sum_o v_o * tanh(E_o + q_o) on DVE, then one small
        # matmul per 512-block contracts the 128 partitions into psS.
        # The chain uses only 2x-capable DVE ops (tensor_scalar muls +
        # f16 tensor_tensor adds; scalar_tensor_tensor has no 2x uop and
        # ran at 1 elem/cycle). The z-matmul for s is emitted during
        # s+1's o=2 main matmuls so the in-order PE never waits on the
        # DVE chain it just scheduled.
        def emit_zmm(z, s, b):
            for h2 in range(SUP // BLK):
                c = 2 * s + h2
                nc.tensor.matmul(
                    psS16[:],
                    lhsT=onesw_sb[:, 15 - c : 31 - c],
                    rhs=z[:, h2 * BLK : (h2 + 1) * BLK],
                    start=(s == 0 and h2 == 0),
                    stop=False,
                )

        def emit_vdot(th, s, o, h2s, stop=False):
            # direct PE v-dot (used for the last superblock so the tail
            # does not wait on the z chain)
            for h2 in h2s:
                c = 2 * s + h2
                nc.tensor.matmul(
                    psS16[:],
                    lhsT=vwin_sb[o][:, 15 - c : 31 - c],
                    rhs=th[:, h2 * BLK : (h2 + 1) * BLK],
                    start=False,
                    stop=(stop and h2 == h2s[-1]),
                )

        pending = None
        for s in range(NSUP):
            b = s // 2
            last_sup = s == NSUP - 1
            z = None
            for o in range(OC):
                last_o = last_sup and o == OC - 1
                psE = psE_pool.tile([P, SUP], F32, tag="E")
                for h2 in range(SUP // BLK):
                    for kp in range(KP):
                        nc.tensor.matmul(
                            psE[:, h2 * BLK : (h2 + 1) * BLK],
                            lhsT=wetp_sb[
                                :, 2 * kp : 2 * kp + 2, o * P : (o + 1) * P
                            ],
                            rhs=enc_sb[s][
                                :, 2 * kp : 2 * kp + 2, h2 * BLK : (h2 + 1) * BLK
                            ],
                            start=(kp == 0),
                            stop=(kp == KP - 1),
                            perf_mode=DR,
                        )
                if pending is not None and o == 2:
                    emit_zmm(*pending)
                    pending = None
                if last_sup and 0 < o:
                    emit_vdot(prev_th, s, o - 1, [0, 1])
                th = tanh_pool.tile([P, SUP], F16, tag="tanh", bufs=10)
                qcol = q_sb[:, o * NB + b : o * NB + b + 1]
                if last_o:
                    # finer tail: tanh + v-dot per 512-block so the last
                    # score matmul starts one half-tile earlier
                    for h2 in range(SUP // BLK):
                        nc.scalar.activation(
                            th[:, h2 * BLK : (h2 + 1) * BLK],
                            psE[:, h2 * BLK : (h2 + 1) * BLK],
                            AF.Tanh,
                            bias=qcol,
                            scale=1.0 / WS,
                        )
                        emit_vdot(th, s, o, [h2], stop=(h2 == 1))
                else:
                    nc.scalar.activation(
                        th[:], psE[:], AF.Tanh, bias=qcol, scale=1.0 / WS
                    )
                if not last_sup:
                    if o == 0:
                        z = tanh_pool.tile([P, SUP], F16, tag="z", bufs=4)
                        nc.vector.tensor_scalar_mul(z[:], th[:], vp_sb[:, 0:1])
                    else:
                        nc.vector.scalar_tensor_tensor(
                            z[:],
                            th[:],
                            vp_sb[:, o : o + 1],
                            z[:],
                            op0=mybir.AluOpType.mult,
                            op1=mybir.AluOpType.add,
                        )
                prev_th = th
            if not last_sup:
                pending = (z, s, b)

        # softmax on the (16, 512) layout; scores are bounded (|s| <=
        # ||v||_1) so no max-subtraction is needed in f32
        ex16 = sm_pool.tile([16, BLK], F32, tag="ex16")
        sums16 = sm_pool.tile([16, 1], F32, tag="sums16")
        nc.scalar.activation(ex16[:], psS16[:], AF.Exp, accum_out=sums16[:])
        # per-batch sums: contract the 4 j-rows of each batch on PE
        psT = psS_pool.tile([NB, 1], F32, tag="t", name="psT")
        nc.tensor.matmul(
            psT[:], lhsT=selb2_sb[:], rhs=sums16[:], start=True, stop=True
        )
        rec4 = sm_pool.tile([NB, 1], F32, tag="rec4")
        nc.vector.reciprocal(rec4[:], psT[:])
        # broadcast 1/sum back to the 16 rows
        psB = psS_pool.tile([16, 1], F32, tag="t", name="psB")
        nc.tensor.matmul(
            psB[:], lhsT=selb_sb[:], rhs=rec4[:], start=True, stop=True
        )
        probs16 = sm_pool.tile([16, BLK], F32, tag="probs16")
        nc.vector.tensor_scalar_mul(probs16[:], ex16[:], psB[:, 0:1])
        nc.sync.dma_start(
            out[:, :].rearrange("b (j t) -> (b j) t", j=4), probs16[:]
        )

    nc.compile()
    return nc


_NC = None


def _get_nc():
    global _NC
    if _NC is None:
        _NC = _build()
    return _NC


def _shard_inputs(hidden, encoder_outputs, W_attn, b_attn, v):
    hidden = np.asarray(hidden, dtype=np.float32)
    encoder_outputs = np.asarray(encoder_outputs, dtype=np.float32)
    W_attn = np.asarray(W_attn, dtype=np.float32)
    b_attn = np.asarray(b_attn, dtype=np.float32)
    v = np.asarray(v, dtype=np.float32)

    # wetp/whtp8[p, k*H+o] = 32 * W^T[k*128+p, o] (e4m3)
    wet_t = (W_attn[:, H:].T * WS).astype(NP_F8)  # (H, H) [h, o]
    wht_t = (W_attn[:, :H].T * WS).astype(NP_F8)
    wetp = np.ascontiguousarray(
        wet_t.reshape(KC, P, H).transpose(1, 0, 2).reshape(P, KC * H)
    )
    whtp8 = np.ascontiguousarray(
        wht_t.reshape(KC, P, H).transpose(1, 0, 2).reshape(P, KC * H)
    )

    # packed head block, f32 view (P, 132) / f16 view (P, 264)
    headc = np.zeros((P, 132), dtype=np.float32)
    c16 = headc[:, 0:128].view(np.float16)  # (P, 256)
    headc[:, 0:16] = np.repeat(b_attn.reshape(OC, P).T, NB, axis=1)
    headc[:, 16:20] = v.reshape(OC, P).T
    for b in range(NB):
        for j in range(NB):
            headc[NB * b + j, 20 + b] = 1.0  # selb2 (rows 0:16)
            headc[b, 24 + NB * b + j] = 1.0  # selb (rows 0:4)
    c16[:, 80 + 15] = np.float16(1.0)  # onesw: ones at f16-col 15
    vrT = v.reshape(OC, P).T.astype(np.float16)  # (P, OC)
    for o in range(OC):
        c16[:, 128 + 32 * o + 15] = vrT[:, o]  # vwin sliding windows
    c16[:, 159] = np.float16(1.0 / WS)  # q descale constant

    # (H, B, T) so per-core slices are cheap views before the copy
    enc_hbt = np.transpose(encoder_outputs, (2, 1, 0))
    in_maps = []
    for c in range(NCORES):
        b0 = c * NB
        # enct[p, s, k*SUP+j] = enc^T[k*128+p, s*SUP+j] (e4m3)
        x = enc_hbt[:, b0 : b0 + NB, :].reshape(H, ROWS).astype(NP_F8)
        enct = np.ascontiguousarray(
            x.reshape(KC, P, NSUP, SUP).transpose(1, 2, 0, 3)
        ).reshape(P, NSUP, KC * SUP)
        hh = headc.copy()
        # hid8[p, k*NB+b] = e4m3(hidden[b, k*128+p])
        hid8 = np.ascontiguousarray(
            hidden[0, b0 : b0 + NB, :].T.astype(NP_F8).reshape(KC, P, NB)
            .transpose(1, 0, 2).reshape(P, KC * NB)
        )
        hh[:, 128:132] = hid8.view(np.float32)
        in_maps.append(
            {"enct": enct, "wetp": wetp, "whtp8": whtp8, "head": hh}
        )
    return in_maps


def kernel(hidden, encoder_outputs, W_attn, b_attn, v):
    nc = _get_nc()
    in_maps = _shard_inputs(hidden, encoder_outputs, W_attn, b_attn, v)
    res = bass_utils.run_bass_kernel_spmd(
        nc, in_maps, core_ids=list(range(NCORES))
    )
    outs = [res.results[c]["out"] for c in range(NCORES)]  # each (NB, T)
    full = np.concatenate(outs, axis=0)  # (B, T)
    return full[:, None, :].astype(np.float32)  # (B, 1, T)
